# revision 15
# baseline (speedup 1.0000x reference)
"""Trainium2 Bass kernel for the ESN (echo state network) forward scan.

  x_{t+1} = (1-a) x_t + a tanh(u_t + x_t @ W),  a = 0.5
  U = einsum('bit,in->tbn', Input, W_in);  out X[b,n,t] = x_{t+1}[b,n]

Sharding: data-parallel over batch (B=64 -> 8 cores x 8 batches).
W, W_in replicated; no collectives. Each core runs the full T=2000 scan
for its 8 batches and writes its [8, 1024, 2000] output slice.

Per-core per-step data flow (all on-chip, only X streamed out):
  PE:  z partials (PSUM, col-tiled 4x): group j of bank h accumulates
       k-tiles {j, j+4} at psum partitions 32j..32j+8 concurrently via
       tile_position=(0,32j) -- 4 matmuls stream W through 4 XBUSes at
       once, so a 4-matmul round costs ~1 matmul of wall time. u is a
       K=16 matmul col-tiled into groups 0 (bank0) / 1 (bank1).
  ACT/DVE: psum -> zp16 fp16 copies (split across both engines)
  PE:  sel matmul fuses the cross-group sum + transpose:
       zT[nloc, b] = sum_p zp16[p, nloc] * sel[p, b], sel[32j+b, b] = 1
  ACT: hT = tanh(zT)
  DVE: s = xT + hT; xT' = 0.5 s; obuf[:, :, t] = 0.5 s   (x_{t+1})
Output chunks of Tc steps buffered in SBUF, DMA'd as [128, Tc]-contiguous
blocks into X[b, 128g:128g+128, t0:t0+Tc].
"""

import copy
import math
import os
import numpy as np

import concourse.bass as bass
import concourse.mybir as mybir
import concourse.tile as tile
from concourse.bass import ds
from concourse.bass_utils import run_bass_kernel_spmd

FP32 = mybir.dt.float32
FP16 = mybir.dt.float16

ALPHA = 0.5
N_CORES = 8
B, N_IN, T, N = 64, 16, 2000, 1024
TC = 100  # steps buffered per output chunk

LAST_EXEC_NS = None
LAST_TRACE = None
_CACHED_NC = None


def _split_excess_waits(nc, limit=1):
    """The walrus build in this container rejects instructions carrying more
    than one sem wait; hoist extra waits onto same-engine NoOps."""
    import bass_rust
    for f in nc.m.functions:
        for bb in f.blocks:
            new_insts = []
            for ins in bb.instructions:
                si = ins.sync_info
                if si is not None and si.on_wait and len(si.on_wait) > limit:
                    waits = list(si.on_wait)
                    head, tail = waits[:-limit], waits[-limit:]
                    for j, w in enumerate(head):
                        c = bass_rust.InstNoOp(name=f"{ins.name}-w{j}")
                        c.engine = ins.engine
                        c.sync_info = mybir.SyncInfo(on_wait=[w], on_update=[])
                        new_insts.append(c)
                    si.on_wait = tail
                new_insts.append(ins)
            bb.instructions = new_insts
    return nc


def _build_nc(n=N, t_total=T, tc_steps=TC, n_in=N_IN, bc=B // N_CORES,
              split_waits=True):
    G = n // 128
    n_chunks = t_total // tc_steps
    NB = (n + 511) // 512
    nb_sizes = [min(512, n - 512 * i) for i in range(NB)]

    assert NB == 2 and G % 2 == 0
    Gh = G // 2  # g-tiles per n-half

    nc = bass.Bass()
    sel_dram = nc.dram_tensor("sel", [128, 8], FP16, kind="ExternalInput")
    w_dram = nc.dram_tensor("w", [128, G * n], FP16, kind="ExternalInput")
    win_dram = nc.dram_tensor("win", [n_in, n], FP16, kind="ExternalInput")
    inpT_dram = nc.dram_tensor("inpT", [n_in, t_total, bc], FP16,
                               kind="ExternalInput")
    x_dram = nc.dram_tensor("xout", [bc, n, t_total], FP32,
                            kind="ExternalOutput")
    x_dram_r = x_dram.rearrange("b (g p) t -> p g b t", p=128)

    with tile.TileContext(nc) as tc:
        with (
            tc.tile_pool(name="const", bufs=1) as const_pool,
            tc.tile_pool(name="state", bufs=1) as state_pool,
            tc.tile_pool(name="work", bufs=3) as work_pool,
            tc.tile_pool(name="obuf", bufs=2) as obuf_pool,
            tc.tile_pool(name="inp", bufs=2) as inp_pool,
            tc.tile_pool(name="psum", bufs=2, space="PSUM") as psum_pool,
            tc.tile_pool(name="psumS", bufs=1, space="PSUM") as psum_static,
        ):
            w_sb = const_pool.tile([128, G * n], FP16)
            nc.sync.dma_start(w_sb[:, :], w_dram[:, :])
            win_sb = const_pool.tile([n_in, n], FP16)
            nc.sync.dma_start(win_sb[:, :], win_dram[:, :])
            sel_sb = const_pool.tile([128, 8], FP16)
            nc.sync.dma_start(sel_sb[:, :], sel_dram[:, :])
            zero16 = const_pool.tile([128, 512], FP16)
            nc.vector.memset(zero16[:, :], 0.0)

            # 4 static psum banks for the col-tiled z partials (ping-pong per
            # step); zero-filled once so never-written partition rows stay
            # finite (sel rows are 0 there, and PE treats 0*garbage as NaN
            # if garbage were NaN)
            zpsS = [[psum_static.tile([128, 512], FP32, name=f"zps_{h_}_{b_}")
                     for b_ in range(2)] for h_ in range(2)]
            for h_ in range(2):
                for b_ in range(2):
                    nc.tensor.matmul(
                        zpsS[h_][b_][:, :], zero16[:, 0:128], zero16[:, :],
                        start=True, stop=True, skip_group_check=True)

            # Single state tensor, split per n-half for dep granularity:
            #   s16[.]  fp16 scaled state s_t = 2 x_t -> matmul operand
            #           (the 0.5 leak is folded into W host-side)
            # Update: s_{t+1} = 0.5 s_t + tanh(z_t), one DVE op per half.
            # Output: x_{t+1} = 0.5 s_{t+1}, one GpSimd op per half.
            s16s = [[state_pool.tile([128, Gh * 8], FP16, name=f"s16_{b_}_{h_}")
                     for h_ in range(2)] for b_ in range(2)]
            for b_ in range(2):
                for h_ in range(2):
                    nc.vector.memset(s16s[b_][h_][:, :], 0.0)

            def chunk_body(ci):
                inp_sb = inp_pool.tile([n_in, tc_steps * bc], FP16)
                nc.sync.dma_start(
                    inp_sb[:, :], inpT_dram[:, ds(ci * tc_steps, tc_steps), :])
                obuf = obuf_pool.tile([128, G * 8 * tc_steps], FP32)
                obuf_r = obuf[:, :].rearrange(
                    "p (g b t) -> p g b t", g=G, b=8, t=tc_steps)

                def emit_u(t):
                    # u for step t opens col group h of bank h; the two
                    # K=16 matmuls run concurrently (distinct col groups)
                    zp = [zpsS[h][t % 2] for h in range(2)]
                    for h in range(2):
                        nc.tensor.matmul(
                            zp[h][32 * h: 32 * h + 8, :],
                            inp_sb[:, t * bc: (t + 1) * bc],
                            win_sb[:, 512 * h: 512 * (h + 1)],
                            start=True, stop=False, skip_group_check=True,
                            tile_position=(0, 32 * h),
                        )
                    return zp

                zps_cur = emit_u(0)
                for t in range(tc_steps):
                    s16, s16_n = s16s[t % 2], s16s[(t + 1) % 2]
                    zps = zps_cur
                    # round order [b0r0, b1r0, b0r1, b1r1]: both banks'
                    # r0 (state half 0) first, so next step's h1 state is
                    # not needed until +2 rounds into the step
                    for r in range(2):
                        for h in range(2):
                            base = 512 * h
                            for j in range(4):
                                # col group j accumulates k-tiles {j, j+4}
                                # at psum partitions 32j..32j+8; the 4
                                # matmuls of a round stream concurrently.
                                # Group h of bank h was opened by emit_u.
                                g = 4 * r + j
                                nc.tensor.matmul(
                                    zps[h][32 * j: 32 * j + 8, :],
                                    s16[g // Gh][:, (g % Gh) * 8:
                                                 (g % Gh) * 8 + 8],
                                    w_sb[:, g * n + base: g * n + base + 512],
                                    start=(r == 0 and j != h),
                                    stop=(r == 1),
                                    skip_group_check=True,
                                    tile_position=(0, 32 * j),
                                )
                    # psum -> fp16 SBUF copies: one tile per (bank, engine)
                    # half so ACT and DVE run concurrently with no false
                    # WAW, and per-engine FIFO order is [copies..., then
                    # tanh/stt] so no copy blocks a critical op.
                    zp16s = []
                    for h in range(2):
                        zpA = work_pool.tile([128, 256], FP16, tag=f"zpA{h}",
                                             name=f"zpA{h}")
                        zpB = work_pool.tile([128, 256], FP16, tag=f"zpB{h}",
                                             name=f"zpB{h}")
                        nc.scalar.copy(zpA[:, :], zps[h][:, 0:256])
                        nc.vector.tensor_scalar_mul(
                            zpB[:, :], zps[h][:, 256:512], 1.0)
                        zp16s.append((zpA, zpB))
                    for h in range(2):
                        # strip-reduce + transpose fused on PE:
                        # zT[nloc, b] = sum_p zp16[p, nloc] * sel[p, b]
                        zTp = psum_pool.tile([128, Gh * 8], FP32,
                                             tag=f"zT{h}", name=f"zTp{h}")
                        for c in range(4):
                            src = zp16s[h][c // 2]
                            nc.tensor.matmul(
                                zTp[:, 8 * c: 8 * c + 8],
                                src[:, 128 * (c % 2): 128 * (c % 2) + 128],
                                sel_sb[:, :],
                                start=(c == 0), stop=(c == 3),
                                skip_group_check=True,
                            )
                        hT = work_pool.tile([128, Gh * 8], FP16, tag=f"hT{h}",
                                            name=f"hT{h}")
                        nc.scalar.activation(
                            hT[:, :], zTp[:, :],
                            mybir.ActivationFunctionType.Tanh)
                        # critical: s_{t+1} = 0.5 s_t + h in one DVE op
                        nc.vector.scalar_tensor_tensor(
                            s16_n[h][:, :], s16[h][:, :], ALPHA, hT[:, :],
                            mybir.AluOpType.mult, mybir.AluOpType.add)
                        # off the critical path: x_{t+1} = 0.5 s_{t+1}
                        s16n_r = s16_n[h][:, :].rearrange(
                            "p (g b) -> p g b", g=Gh, b=8)
                        nc.gpsimd.tensor_scalar_mul(
                            obuf_r[:, Gh * h: Gh * (h + 1), :, t],
                            s16n_r[:, :, :], ALPHA)
                    if t + 1 < tc_steps:
                        # u for the next step, emitted after the sel matmuls
                        # so it fills the PE gap while stt produces s16
                        zps_cur = emit_u(t + 1)

                for g in range(G):
                    nc.sync.dma_start(
                        x_dram_r[:, g, :, ds(ci * tc_steps, tc_steps)],
                        obuf_r[:, g, :, :],
                    )

            with tc.For_i(0, n_chunks, 1) as i:
                chunk_body(i)

    if split_waits:
        _split_excess_waits(nc)
    return nc


def kernel(Input, W_in, W):
    """Full inputs in, full output out. Shards batch over 8 NeuronCores."""
    global LAST_EXEC_NS, _CACHED_NC
    Input = np.ascontiguousarray(np.asarray(Input, dtype=np.float32))
    W_in = np.ascontiguousarray(np.asarray(W_in, dtype=np.float32))
    W = np.ascontiguousarray(np.asarray(W, dtype=np.float32))
    Bf, n_in, t_total = Input.shape
    n = W.shape[0]
    G = n // 128
    bc = Bf // N_CORES

    tc_steps = TC if t_total % TC == 0 else max(
        d for d in range(1, min(TC, t_total) + 1) if t_total % d == 0)
    if _CACHED_NC is None:
        _CACHED_NC = _build_nc(n=n, t_total=t_total, tc_steps=tc_steps,
                               n_in=n_in, bc=bc)
    nc = _CACHED_NC

    # leak factor folded into W: matmul operand is s = x + h = 2x, so W/2
    w_r = np.ascontiguousarray(
        (ALPHA * W).reshape(G, 128, n).transpose(1, 0, 2).reshape(128, G * n)
    ).astype(np.float16)
    win16 = W_in.astype(np.float16)
    sel = np.zeros((128, 8), dtype=np.float16)
    for j in range(4):
        for b_ in range(8):
            sel[32 * j + b_, b_] = 1.0
    in_maps = []
    for c in range(N_CORES):
        inpT = np.ascontiguousarray(
            Input[c * bc:(c + 1) * bc].transpose(1, 2, 0)).astype(np.float16)
        in_maps.append({"w": w_r, "win": win16, "inpT": inpT, "sel": sel})

    trace = bool(int(os.environ.get("ESN_TRACE", "0")))
    res = run_bass_kernel_spmd(
        nc, in_maps, core_ids=list(range(N_CORES)), trace=trace)
    LAST_EXEC_NS = res.exec_time_ns
    global LAST_TRACE
    LAST_TRACE = res.instructions_and_trace

    out = np.concatenate([res.results[c]["xout"] for c in range(N_CORES)],
                         axis=0)
    return np.ascontiguousarray(out.astype(np.float32))



# revision 25
# speedup vs baseline: 1.3764x; 1.3764x over previous
"""Trainium2 Bass kernel for the ESN (echo state network) forward scan.

  x_{t+1} = (1-a) x_t + a tanh(u_t + x_t @ W),  a = 0.5
  U = einsum('bit,in->tbn', Input, W_in);  out X[b,n,t] = x_{t+1}[b,n]

Sharding: data-parallel over batch (B=64 -> 8 cores x 8 batches).
W, W_in replicated; no collectives. Each core runs the full T=2000 scan
for its 8 batches and writes its [8, 1024, 2000] output slice.

Per-core per-step data flow (all on-chip, only X streamed out):
  PE:  z partials (PSUM, col-tiled 4x): group j of bank h accumulates
       k-tiles {j, j+4} at psum partitions 32j..32j+8 concurrently via
       tile_position=(0,32j) -- 4 matmuls stream W through 4 XBUSes at
       once, so a 4-matmul round costs ~1 matmul of wall time. u is a
       K=16 matmul col-tiled into groups 0 (bank0) / 1 (bank1).
  ACT/DVE: psum -> zp16 fp16 copies (split across both engines)
  PE:  sel matmul fuses the cross-group sum + transpose:
       zT[nloc, b] = sum_p zp16[p, nloc] * sel[p, b], sel[32j+b, b] = 1
  ACT: hT = tanh(zT)
  DVE: s = xT + hT; xT' = 0.5 s; obuf[:, :, t] = 0.5 s   (x_{t+1})
Output chunks of Tc steps buffered in SBUF, DMA'd as [128, Tc]-contiguous
blocks into X[b, 128g:128g+128, t0:t0+Tc].
"""

import copy
import math
import os
import numpy as np

import concourse.bass as bass
import concourse.mybir as mybir
import concourse.tile as tile
from concourse.bass import ds
from concourse.bass_utils import run_bass_kernel_spmd

FP32 = mybir.dt.float32
FP16 = mybir.dt.float16

ALPHA = 0.5
N_CORES = 8
B, N_IN, T, N = 64, 16, 2000, 1024
TC = 100  # steps buffered per output chunk

LAST_EXEC_NS = None
LAST_TRACE = None
_CACHED_NC = None


def _split_excess_waits(nc, limit=1):
    """The walrus build in this container rejects instructions carrying more
    than one sem wait; hoist extra waits onto same-engine NoOps."""
    import bass_rust
    for f in nc.m.functions:
        for bb in f.blocks:
            new_insts = []
            for ins in bb.instructions:
                si = ins.sync_info
                if si is not None and si.on_wait and len(si.on_wait) > limit:
                    waits = list(si.on_wait)
                    head, tail = waits[:-limit], waits[-limit:]
                    for j, w in enumerate(head):
                        c = bass_rust.InstNoOp(name=f"{ins.name}-w{j}")
                        c.engine = ins.engine
                        c.sync_info = mybir.SyncInfo(on_wait=[w], on_update=[])
                        new_insts.append(c)
                    si.on_wait = tail
                new_insts.append(ins)
            bb.instructions = new_insts
    return nc


def _prune_pe_sem_updates(nc):
    """PE posts a serialized EVT_SEM write per matmul (~26ns each); with 26
    matmuls/step this delays the bank-close increments consumers wait on by
    ~600ns.  Drop PE counting-sem increments nobody waits for, folding their
    value into the next kept increment so every waited-on threshold is
    reached at the same instruction as before."""
    # collect every (sem id -> waited values) across the module; sems with
    # any register-valued or non-ge wait are excluded from pruning entirely
    waited = {}
    unsafe = set()
    for f in nc.m.functions:
        for bb in f.blocks:
            for ins in bb.instructions:
                si = ins.sync_info
                if si is None:
                    continue
                for w in si.on_wait:
                    if w.wait_mode == "sem-ge-imm" and w.wait_reg is None:
                        waited.setdefault(w.id, set()).add(w.wait_value)
                    else:
                        unsafe.add(w.id)
    # only prune sems updated exclusively by PE instructions (the per-engine
    # counting sem); multi-engine sems (barriers) use cross-engine counts
    upd_engines = {}
    for f in nc.m.functions:
        for bb in f.blocks:
            for ins in bb.instructions:
                si = ins.sync_info
                if si is None:
                    continue
                for u in si.on_update:
                    upd_engines.setdefault(u.id, set()).add(ins.engine)
    for sid in list(waited):
        if upd_engines.get(sid, set()) != {mybir.EngineType.PE}:
            unsafe.add(sid)
    for sid in unsafe:
        waited.pop(sid, None)
    for f in nc.m.functions:
        # running original count is function-global (sem values carry from
        # the prologue into the loop body; the per-chunk reset NoOps
        # subtract explicitly and are modeled below)
        run = {}
        for bb in f.blocks:
            pending = {}  # sem id -> removed value to fold into next kept
            last_removed_idx = -1
            for idx, ins in enumerate(bb.instructions):
                si = ins.sync_info
                if si is None:
                    continue
                for u in si.on_update:
                    if u.id not in waited or u.update_reg is not None:
                        continue
                    if u.update_mode in ("sem-sub-imm", "sem-dec"):
                        run[u.id] = run.get(u.id, 0) - u.update_value
                if ins.engine != mybir.EngineType.PE:
                    continue
                keep_ups = []
                changed = False
                for u in si.on_update:
                    if (u.update_mode == "sem-inc"
                            and u.id in waited
                            and u.update_reg is None):
                        lo = run.get(u.id, 0)
                        hi = lo + u.update_value
                        run[u.id] = hi
                        if any(lo < v <= hi for v in waited[u.id]):
                            kept_count[u.id] = kept_count.get(u.id, 0) + 1
                            remap.setdefault(u.id, {})[hi] = (
                                kept_count[u.id] + folded.get(u.id, 0))
                            keep_ups.append(u)
                        else:
                            pending[u.id] = (pending.get(u.id, 0)
                                             + u.update_value)
                            last_removed_idx = idx
                            changed = True
                    else:
                        keep_ups.append(u)
                if changed:
                    si.on_update = keep_ups
            # fold trailing removed counts into a PE NoOp right after the
            # last pruned instruction (before the per-chunk sem resets)
            if any(pending.values()):
                import bass_rust
                c = bass_rust.InstNoOp(name=f"semfold_{id(bb)}")
                c.engine = mybir.EngineType.PE
                ups = []
                for sid, val in pending.items():
                    if val:
                        ups.append(mybir.SyncUpdate(
                            sync_type="semaphore", id=sid,
                            ant_name=f"fold_{sid}",
                            update_mode="sem-add-imm", update_value=val,
                            update_reg=None))
                c.sync_info = mybir.SyncInfo(on_wait=[], on_update=ups)
                insts = bb.instructions
                insts.insert(last_removed_idx + 1, c)
                bb.instructions = insts
    return nc


def _build_nc(n=N, t_total=T, tc_steps=TC, n_in=N_IN, bc=B // N_CORES,
              split_waits=True):
    G = n // 128
    n_chunks = t_total // tc_steps
    NB = (n + 511) // 512
    nb_sizes = [min(512, n - 512 * i) for i in range(NB)]

    assert NB == 2 and G % 2 == 0
    Gh = G // 2  # g-tiles per n-half

    nc = bass.Bass()
    sel_dram = nc.dram_tensor("sel", [128, 8], FP16, kind="ExternalInput")
    w_dram = nc.dram_tensor("w", [128, G * n], FP16, kind="ExternalInput")
    win_dram = nc.dram_tensor("win", [n_in, n], FP16, kind="ExternalInput")
    inpT_dram = nc.dram_tensor("inpT", [n_in, t_total, bc], FP16,
                               kind="ExternalInput")
    x_dram = nc.dram_tensor("xout", [bc, n, t_total], FP32,
                            kind="ExternalOutput")
    x_dram_r = x_dram.rearrange("b (g p) t -> p g b t", p=128)

    with tile.TileContext(nc) as tc:
        with (
            tc.tile_pool(name="const", bufs=1) as const_pool,
            tc.tile_pool(name="state", bufs=1) as state_pool,
            tc.tile_pool(name="work", bufs=3) as work_pool,
            tc.tile_pool(name="obuf", bufs=2) as obuf_pool,
            tc.tile_pool(name="inp", bufs=2) as inp_pool,
            tc.tile_pool(name="psum", bufs=2, space="PSUM") as psum_pool,
            tc.tile_pool(name="psumS", bufs=1, space="PSUM") as psum_static,
        ):
            w_sb = const_pool.tile([128, G * n], FP16)
            nc.sync.dma_start(w_sb[:, :], w_dram[:, :])
            win_sb = const_pool.tile([n_in, n], FP16)
            nc.sync.dma_start(win_sb[:, :], win_dram[:, :])
            sel_sb = const_pool.tile([128, 8], FP16)
            nc.sync.dma_start(sel_sb[:, :], sel_dram[:, :])
            zero16 = const_pool.tile([128, 512], FP16)
            nc.vector.memset(zero16[:, :], 0.0)

            # 4 static psum banks for the col-tiled z partials (ping-pong per
            # step); zero-filled once so never-written partition rows stay
            # finite (sel rows are 0 there, and PE treats 0*garbage as NaN
            # if garbage were NaN)
            zpsS = [[psum_static.tile([128, 512], FP32, name=f"zps_{h_}_{b_}")
                     for b_ in range(2)] for h_ in range(2)]
            for h_ in range(2):
                for b_ in range(2):
                    nc.tensor.matmul(
                        zpsS[h_][b_][:, :], zero16[:, 0:128], zero16[:, :],
                        start=True, stop=True, skip_group_check=True)

            # Single state tensor, split per n-half for dep granularity:
            #   s16[.]  fp16 scaled state s_t = 2 x_t -> matmul operand
            #           (the 0.5 leak is folded into W host-side)
            # Update: s_{t+1} = 0.5 s_t + tanh(z_t), one DVE op per half.
            # Output: x_{t+1} = 0.5 s_{t+1}, one GpSimd op per half.
            s16s = [[state_pool.tile([128, Gh * 8], FP16, name=f"s16_{b_}_{h_}")
                     for h_ in range(2)] for b_ in range(2)]
            for b_ in range(2):
                for h_ in range(2):
                    nc.vector.memset(s16s[b_][h_][:, :], 0.0)

            def chunk_body(ci):
                inp_sb = inp_pool.tile([n_in, tc_steps * bc], FP16)
                nc.sync.dma_start(
                    inp_sb[:, :], inpT_dram[:, ds(ci * tc_steps, tc_steps), :])
                obuf = obuf_pool.tile([128, G * 8 * tc_steps], FP32)
                obuf_r = obuf[:, :].rearrange(
                    "p (g b t) -> p g b t", g=G, b=8, t=tc_steps)

                def emit_u(t):
                    # u for step t opens col group h of bank h; the two
                    # K=16 matmuls run concurrently (distinct col groups)
                    zp = [zpsS[h][t % 2] for h in range(2)]
                    for h in range(2):
                        nc.tensor.matmul(
                            zp[h][32 * h: 32 * h + 8, :],
                            inp_sb[:, t * bc: (t + 1) * bc],
                            win_sb[:, 512 * h: 512 * (h + 1)],
                            start=True, stop=False, skip_group_check=True,
                            tile_position=(0, 32 * h),
                        )
                    return zp

                zps_cur = emit_u(0)
                for t in range(tc_steps):
                    s16, s16_n = s16s[t % 2], s16s[(t + 1) % 2]
                    zps = zps_cur
                    # bank0's two rounds first so it closes earliest and
                    # its transpose chain starts as soon as possible
                    for h in range(2):
                        for r in range(2):
                            base = 512 * h
                            for j in range(4):
                                # col group j accumulates k-tiles {j, j+4}
                                # at psum partitions 32j..32j+8; the 4
                                # matmuls of a round stream concurrently.
                                # Group h of bank h was opened by emit_u.
                                g = 4 * r + j
                                nc.tensor.matmul(
                                    zps[h][32 * j: 32 * j + 8, :],
                                    s16[g // Gh][:, (g % Gh) * 8:
                                                 (g % Gh) * 8 + 8],
                                    w_sb[:, g * n + base: g * n + base + 512],
                                    start=(r == 0 and j != h),
                                    stop=(r == 1),
                                    skip_group_check=True,
                                    tile_position=(0, 32 * j),
                                )
                    # psum -> fp16 SBUF copies: bank0 wholly on ACT, bank1
                    # wholly on DVE, so they run concurrently and each
                    # engine's FIFO is [copy, then tanh/stt] with no
                    # copy blocking a critical op.
                    zp16s = []
                    for h in range(2):
                        zp16 = work_pool.tile([128, 512], FP16, tag=f"zp{h}",
                                              name=f"zp{h}")
                        if h == 0:
                            nc.scalar.copy(zp16[:, :], zps[h][:, :])
                        else:
                            nc.vector.tensor_scalar_mul(
                                zp16[:, :], zps[h][:, :], 1.0)
                        zp16s.append(zp16)
                    for h in range(2):
                        # strip-reduce + transpose fused on PE:
                        # zT[nloc, b] = sum_p zp16[p, nloc] * sel[p, b]
                        zTp = psum_pool.tile([128, Gh * 8], FP32,
                                             tag=f"zT{h}", name=f"zTp{h}")
                        for c in range(4):
                            nc.tensor.matmul(
                                zTp[:, 8 * c: 8 * c + 8],
                                zp16s[h][:, 128 * c: 128 * c + 128],
                                sel_sb[:, :],
                                start=(c == 0), stop=(c == 3),
                                skip_group_check=True,
                            )
                        hT = work_pool.tile([128, Gh * 8], FP16, tag=f"hT{h}",
                                            name=f"hT{h}")
                        nc.scalar.activation(
                            hT[:, :], zTp[:, :],
                            mybir.ActivationFunctionType.Tanh)
                        # critical: s_{t+1} = 0.5 s_t + h in one DVE op
                        nc.vector.scalar_tensor_tensor(
                            s16_n[h][:, :], s16[h][:, :], ALPHA, hT[:, :],
                            mybir.AluOpType.mult, mybir.AluOpType.add)
                        # off the critical path: x_{t+1} = 0.5 s_{t+1}
                        s16n_r = s16_n[h][:, :].rearrange(
                            "p (g b) -> p g b", g=Gh, b=8)
                        nc.gpsimd.tensor_scalar_mul(
                            obuf_r[:, Gh * h: Gh * (h + 1), :, t],
                            s16n_r[:, :, :], ALPHA)
                    if t + 1 < tc_steps:
                        # u for the next step, emitted after the sel matmuls
                        # so it fills the PE gap while stt produces s16
                        zps_cur = emit_u(t + 1)

                for g in range(G):
                    nc.sync.dma_start(
                        x_dram_r[:, g, :, ds(ci * tc_steps, tc_steps)],
                        obuf_r[:, g, :, :],
                    )

            with tc.For_i(0, n_chunks, 1) as i:
                chunk_body(i)

    if os.environ.get("ESN_PRUNE_SEMS", "0") == "1":
        _prune_pe_sem_updates(nc)
    if split_waits:
        _split_excess_waits(nc)
    return nc


def kernel(Input, W_in, W):
    """Full inputs in, full output out. Shards batch over 8 NeuronCores."""
    global LAST_EXEC_NS, _CACHED_NC
    Input = np.ascontiguousarray(np.asarray(Input, dtype=np.float32))
    W_in = np.ascontiguousarray(np.asarray(W_in, dtype=np.float32))
    W = np.ascontiguousarray(np.asarray(W, dtype=np.float32))
    Bf, n_in, t_total = Input.shape
    n = W.shape[0]
    G = n // 128
    bc = Bf // N_CORES

    tc_steps = TC if t_total % TC == 0 else max(
        d for d in range(1, min(TC, t_total) + 1) if t_total % d == 0)
    if _CACHED_NC is None:
        _CACHED_NC = _build_nc(n=n, t_total=t_total, tc_steps=tc_steps,
                               n_in=n_in, bc=bc)
    nc = _CACHED_NC

    # leak factor folded into W: matmul operand is s = x + h = 2x, so W/2
    w_r = np.ascontiguousarray(
        (ALPHA * W).reshape(G, 128, n).transpose(1, 0, 2).reshape(128, G * n)
    ).astype(np.float16)
    win16 = W_in.astype(np.float16)
    sel = np.zeros((128, 8), dtype=np.float16)
    for j in range(4):
        for b_ in range(8):
            sel[32 * j + b_, b_] = 1.0
    in_maps = []
    for c in range(N_CORES):
        inpT = np.ascontiguousarray(
            Input[c * bc:(c + 1) * bc].transpose(1, 2, 0)).astype(np.float16)
        in_maps.append({"w": w_r, "win": win16, "inpT": inpT, "sel": sel})

    trace = bool(int(os.environ.get("ESN_TRACE", "0")))
    res = run_bass_kernel_spmd(
        nc, in_maps, core_ids=list(range(N_CORES)), trace=trace)
    LAST_EXEC_NS = res.exec_time_ns
    global LAST_TRACE
    LAST_TRACE = res.instructions_and_trace

    out = np.concatenate([res.results[c]["xout"] for c in range(N_CORES)],
                         axis=0)
    return np.ascontiguousarray(out.astype(np.float32))

